# revision 10
# baseline (speedup 1.0000x reference)
"""MixerAttention (GQA + QK-RMSNorm + RoPE + causal) Trainium2 kernel.

Sharding: 8 cores = batch(2) x kv-head(4). Fully local per core — no collectives.
Each core, for its (batch b, kv head h):
  - weights + rope tables + masks load once per NEFF (resident across reps);
    x streams per rep
  - the whole computation is ONE flat software pipeline over all windows of
    all reps: attention heads of window w are interleaved with projection
    waves of window w+1, so cross-engine latency (exp, rope chains, PSUM
    drains) hides under matmuls and there are no body-boundary bubbles
  - projections in bf16, one m-tile per wave, ordered [q0..q3, k, v]; each
    RMSNorm+RoPE chain is emitted one wave late so its copy->square path has
    slack before the PE reaches its ssb matmul
  - V is computed directly in natural (t x DH) layout: x-block stationary,
    Wv^T moving, one PSUM accumulation group per 128-row block (kept
    consecutive — PSUM banks allow one open accumulation group)
  - QK RMSNorm via the ln/exp rsqrt path with the 1/sqrt(DH) scale folded
    into q; RoPE on the DVE in bf16 (2x rate), rsqrt applied after rope
  - causal attention in S^T layout: scores^T (t_k partitions x t_q free) in
    bf16 at 1 col/cycle; exp on ACT; diagonal blocks masked post-exp by a
    0/1 triangle multiply (DVE bf16); exp(S^T) feeds P@V as bf16 moving
  - softmax denominators without per-block ones-matmuls: es blocks are
    pair-summed on GpSimd and quad-summed on DVE (fp32), then one short
    ones-matmul per quad + per diagonal block accumulates into PSUM (~4x
    less PE time than a ones-matmul per block)
  - normalization on-chip (reciprocal_approx_fast), y written fp32
Output per core is y^T (4*128, T); the host reassembles (B, T, H*DH).
"""
import sys

sys.path.insert(0, "/opt/trn_rl_repo")
from contextlib import ExitStack

import numpy as np
import ml_dtypes
import concourse.bacc as bacc
import concourse.mybir as mybir
import concourse.tile as tile
from concourse.bass_utils import run_bass_kernel_spmd

F32 = mybir.dt.float32
F32R = mybir.dt.float32r
BF16 = mybir.dt.bfloat16
NPBF = ml_dtypes.bfloat16
AF = mybir.ActivationFunctionType

B, T, D = 2, 2048, 2048
H, HKV, DH = 16, 4, 128
G = H // HKV                    # q heads per kv head (per core)
EPS = 1.1920928955078125e-07
ROPE_BASE = 10000.0
NCORES = 8

P = 128                         # partitions
HP = P // 2
DCH = D // P                    # 16 d-chunks (contraction)
NT = 4                          # column windows of 512
TC = T // NT                    # 512
EQ = G * DH                     # 512
ETOT = EQ + DH + DH             # 768
QC = 512                        # attention q-chunk == TC
KC = 128                        # attention k-chunk
NKC = T // KC                   # 16
MK, MV = G, G + 1               # m-tile indices of k and v rows

PJB = 2                         # PSUM bufs: pj ring
SPSB = 3                        # PSUM bufs: scores ring  (PJB+SPSB+3 == 8)


class _Pipeline:
    def __init__(self, nc, tc, ctx, su, reps):
        self.nc = nc
        self.su = su
        self.reps = reps
        self.NW = NT * reps
        self.XT = nc.cur_io["xT"]
        self.YT = nc.cur_io["yT"]
        self.finp = ctx.enter_context(tc.tile_pool(name="final", bufs=2))
        self.xp = ctx.enter_context(tc.tile_pool(name="xp", bufs=36))
        self.stg = ctx.enter_context(tc.tile_pool(name="stg", bufs=7))
        self.sp = ctx.enter_context(tc.tile_pool(name="sp", bufs=2))
        self.qsc = ctx.enter_context(tc.tile_pool(name="qsc", bufs=8))
        self.asb = ctx.enter_context(tc.tile_pool(name="asb", bufs=6))
        self.asb2 = ctx.enter_context(tc.tile_pool(name="asb2", bufs=2))
        self.prp = ctx.enter_context(tc.tile_pool(name="prp", bufs=3))
        self.qdp = ctx.enter_context(tc.tile_pool(name="qdp", bufs=2))
        self.cps = ctx.enter_context(
            tc.tile_pool(name="cps", bufs=1, space="PSUM"))
        self.xns = {}
        self.rstate = {}
        self.wstate = {}

    def load_x(self, w, d):
        n = w % NT
        xn = self.xp.tile([P, TC], BF16, tag="xn", name=f"xn_{w}_{d}")
        self.nc.sync.dma_start(out=xn, in_=self.XT[d, :, n * TC : (n + 1) * TC])
        self.xns[(w, d)] = xn

    def rep_state(self, rep):
        if rep not in self.rstate:
            self.rstate[rep] = dict(
                KTr=self.finp.tile([P, T], BF16, tag="KTr", bufs=2,
                                   name=f"KTr_{rep}"),
                Vnat=self.finp.tile([P, NKC, KC], BF16, tag="Vnat", bufs=2,
                                    name=f"Vnat_{rep}"),
            )
        return self.rstate[rep]

    def win_state(self, w):
        if w not in self.wstate:
            self.wstate[w] = dict(stage={}, qtr={})
        return self.wstate[w]

    def wave(self, w, m):
        nc = self.nc
        rep, n = divmod(w, NT)
        rs = self.rep_state(rep)
        ws = self.win_state(w)
        wt = self.su["wt"]
        if m == MV:
            # V directly in natural layout; one accumulation group per
            # 128-row block, kept consecutive within the PSUM bank
            vnps = self.cps.tile([P, TC], F32, tag="pj", bufs=PJB,
                                 name=f"pjv_{w}")
            for jj in range(4):
                for d in range(DCH):
                    nc.tensor.matmul(
                        vnps[:, jj * KC : (jj + 1) * KC],
                        self.xns[(w, d)][:, jj * KC : (jj + 1) * KC],
                        wt[:, d, MV * P : (MV + 1) * P],
                        start=(d == 0),
                        stop=(d == DCH - 1),
                    )
            Vnat = rs["Vnat"]
            for jj in range(4):
                nc.vector.tensor_copy(
                    Vnat[:, 4 * n + jj, :], vnps[:, jj * KC : (jj + 1) * KC]
                )
            return
        psl = self.cps.tile([P, TC], F32, tag="pj", bufs=PJB,
                            name=f"pj_{w}_{m}")
        for d in range(DCH):
            nc.tensor.matmul(
                psl,
                wt[:, d, m * P : (m + 1) * P],
                self.xns[(w, d)],
                start=(d == 0),
                stop=(d == DCH - 1),
            )
        st = self.stg.tile([P, TC], BF16, tag="stage", name=f"st_{w}_{m}")
        nc.vector.tensor_copy(st, psl)
        ws["stage"][m] = st

    def chain(self, w, m):
        nc = self.nc
        rep, n = divmod(w, NT)
        rs = self.rep_state(rep)
        ws = self.win_state(w)
        su = self.su
        src = ws["stage"][m]
        c0 = n * TC
        if m == MK:
            dst, dst0 = rs["KTr"], n * TC
            ln_scale, ln_bias = 1.0 / P, su["bk"]
            label = f"k_{w}"
        else:
            ws["qtr"][m] = self.qsc.tile([P, TC], BF16, tag="qtr",
                                         name=f"qtr_{m}_{w}")
            dst, dst0 = ws["qtr"][m], 0
            ln_scale, ln_bias = 1.0, su["bq"]
            label = f"q{m}_{w}"
        sq = self.sp.tile([P, TC], BF16, tag="sq", name=f"sq_{label}")
        nc.vector.tensor_mul(sq, src, src)
        ssb = self.cps.tile([P, TC], F32, tag="pj", bufs=PJB,
                            name=f"ssb_{label}")
        nc.tensor.matmul(ssb, su["ones_b"], sq, start=True, stop=True)
        lnt = self.sp.tile([P, TC], F32, tag="lnt", name=f"lnt_{label}")
        nc.scalar.activation(lnt, ssb, AF.Ln, scale=ln_scale,
                             bias=ln_bias[:, :])
        rs_t = self.sp.tile([P, TC], BF16, tag="rs", name=f"rs_{label}")
        nc.scalar.activation(rs_t, lnt, AF.Exp, scale=-0.5)
        # rope: dst = (src*C + rot(src)*S) * rs  (copies may shift
        # partitions, tensor-tensor ops may not)
        tmp = self.sp.tile([P, TC], BF16, tag="rtmp", name=f"rtmp_{label}")
        nc.vector.tensor_copy(tmp[0:HP, :], src[HP:P, :])
        nc.vector.tensor_copy(tmp[HP:P, :], src[0:HP, :])
        t1 = self.sp.tile([P, TC], BF16, tag="rt1", name=f"rt1_{label}")
        nc.vector.tensor_mul(t1, src, su["ropeC"][:, c0 : c0 + TC])
        nc.vector.tensor_mul(tmp, tmp, su["ropeS"][:, c0 : c0 + TC])
        nc.vector.tensor_add(t1, t1, tmp)
        nc.vector.tensor_mul(dst[:, dst0 : dst0 + TC], t1, rs_t)

    # chunk c: (wave to emit, chain to emit); chains lag their wave by two
    _SPEC = [(0, None), (1, None), (2, 0), (3, 1), (MK, 2), (MV, 3),
             (None, MK)]

    def proj_chunk(self, w, c):
        if w >= self.NW:
            return
        m, cm = self._SPEC[c]
        if c < 4 and w + 1 < self.NW:
            for d in range(4 * c, 4 * c + 4):
                self.load_x(w + 1, d)
        if m is not None:
            self.wave(w, m)
        if cm is not None:
            self.chain(w, cm)

    def score_block(self, w, g, j):
        nc = self.nc
        su = self.su
        rep, i = divmod(w, NT)
        rs = self.rep_state(rep)
        ws = self.win_state(w)
        KTr, qtr = rs["KTr"], ws["qtr"][g]
        dcol = max(0, j * KC - i * QC)
        sps = self.cps.tile([P, QC], F32, tag="sps", bufs=SPSB,
                            name=f"sps_{g}_{w}_{j}")
        es = self.asb.tile([P, QC], BF16, tag="es", bufs=6,
                           name=f"es_{g}_{w}_{j}")
        nc.tensor.matmul(
            sps[:, dcol:QC],
            KTr[:, j * KC : (j + 1) * KC],
            qtr[:, dcol:QC],
            start=True,
            stop=True,
        )
        nc.scalar.activation(es[:, dcol:QC], sps[:, dcol:QC], AF.Exp)
        if j * KC >= i * QC:  # diagonal: zero the upper triangle
            nc.vector.tensor_mul(
                es[:, dcol : dcol + KC], es[:, dcol : dcol + KC],
                su["tri01"]
            )
        return es

    def attn_prefix(self, w, g, npre):
        # scores j<npre emitted before the interleaved proj wave: their exps
        # compute during the wave, so the PVs in attn_rest never stall
        self._pre = [self.score_block(w, g, j) for j in range(npre)]

    def attn_rest(self, w, g):
        nc = self.nc
        su = self.su
        rep, i = divmod(w, NT)
        rs = self.rep_state(rep)
        Vnat = rs["Vnat"]
        nk = 4 * (i + 1)
        nfull = 4 * i
        yps = self.cps.tile([P, QC], F32, tag="yps", bufs=2,
                            name=f"yps_{g}_{w}")
        rps = self.cps.tile([P, QC], F32, tag="rps", bufs=1,
                            name=f"rps_{g}_{w}")
        es_prev = None
        pair = {}
        quads = []
        for j in range(nk):
            dcol = max(0, j * KC - i * QC)
            if j < len(self._pre):
                es = self._pre[j]
            else:
                es = self.score_block(w, g, j)
            nc.tensor.matmul(
                yps[:, dcol:QC],
                Vnat[:, j, :],
                es[:, dcol:QC],
                start=(j == 0),
                stop=(j == nk - 1),
            )
            if j < nfull:
                # denominator tree: pair on GpSimd, quad on DVE (fp32)
                if j % 2 == 1:
                    pr = self.prp.tile([P, QC], F32R, tag="pair", bufs=3,
                                       name=f"pr_{g}_{w}_{j}")
                    nc.gpsimd.tensor_add(pr, es_prev, es)
                    pair[j // 2] = pr
                else:
                    es_prev = es
                if j % 4 == 3:
                    qd = self.qdp.tile([P, QC], F32R, tag="quad", bufs=2,
                                       name=f"qd_{g}_{w}_{j}")
                    nc.vector.tensor_add(qd, pair[j // 2 - 1], pair[j // 2])
                    quads.append(qd)
            else:
                r = j - nfull
                nc.tensor.matmul(
                    rps[:, dcol:QC],
                    su["ones_b"],
                    es[:, dcol:QC],
                    start=(r == 0),
                    stop=(r == 3),
                )
                # deferred quad matmuls: quad r is ~4 blocks old by now,
                # so its Pool/DVE chain is long done — no PE stall
                if r < len(quads):
                    nc.tensor.matmul(
                        rps, su["ones_r"], quads[r], start=False, stop=False
                    )
        rec = self.asb2.tile([P, QC], F32, tag="rec", name=f"rec_{g}_{w}")
        nc.vector.reciprocal_approx_fast(out=rec, in_=rps)
        yo = self.asb.tile([P, QC], F32, tag="yo", bufs=2, name=f"yo_{g}_{w}")
        nc.vector.tensor_mul(yo, yps, rec)
        nc.sync.dma_start(
            out=self.YT[g * DH : (g + 1) * DH, i * QC : (i + 1) * QC], in_=yo
        )

    def emit(self):
        # prime window 0: x loads + all proj chunks
        for d in range(DCH):
            self.load_x(0, d)
        for c in range(len(self._SPEC)):
            self.proj_chunk(0, c)
        # steady pipeline: attention heads of w interleave proj chunks of w+1
        for w in range(self.NW):
            for g in range(G):
                self.attn_prefix(w, g, min(SPSB, 4 * (w % NT + 1)))
                self.proj_chunk(w + 1, g)
                self.attn_rest(w, g)
            for c in range(G, len(self._SPEC)):
                self.proj_chunk(w + 1, c)


def _setup(nc, tc, ctx):
    """One-time (per NEFF) setup: weights, rope tables, masks, constants."""
    WT = nc.cur_io["wT"]
    RC = nc.cur_io["ropeC"]
    RS_ = nc.cur_io["ropeS"]
    TRI = nc.cur_io["tri01"]

    constp = ctx.enter_context(tc.tile_pool(name="const", bufs=1))
    wp = ctx.enter_context(tc.tile_pool(name="wp", bufs=1))

    wt = wp.tile([P, DCH, ETOT], BF16, tag="wt")
    for lo, hi in [(0, 4), (4, 8), (8, 12), (12, 16)]:
        nc.sync.dma_start(
            out=wt[:, lo:hi, :],
            in_=WT[lo:hi, :, :].rearrange("d p e -> p d e"),
        )
    tri01 = constp.tile([P, KC], BF16, tag="tri01")
    nc.sync.dma_start(out=tri01, in_=TRI[:, :])
    ropeC = constp.tile([P, T], BF16, tag="ropeC")
    nc.sync.dma_start(out=ropeC, in_=RC[:, :])
    ropeS = constp.tile([P, T], BF16, tag="ropeS")
    nc.sync.dma_start(out=ropeS, in_=RS_[:, :])
    ones_f = constp.tile([P, P], F32, tag="ones_f")
    nc.vector.memset(ones_f, 1.0)
    ones_r = constp.tile([P, P], F32R, tag="ones_r")
    nc.vector.tensor_copy(ones_r, ones_f)
    ones_b = constp.tile([P, P], BF16, tag="ones_b")
    nc.vector.tensor_copy(ones_b, ones_f)
    bq = constp.tile([P, 1], F32, tag="bq")
    nc.vector.memset(bq, float(P) * EPS)
    bk = constp.tile([P, 1], F32, tag="bk")
    nc.vector.memset(bk, EPS)
    return dict(wt=wt, tri01=tri01, ropeC=ropeC, ropeS=ropeS, ones_r=ones_r,
                ones_b=ones_b, bq=bq, bk=bk)


def _pin_act_table_set():
    """Restrict the ACT table chooser to natural_log_exp_and_others (which
    holds ln/exp/copy — every function this kernel uses) so the compiled
    stream has one table load instead of one per ln<->exp switch."""
    import concourse.hw_specs as hw_specs

    if getattr(bacc, "_act_tables_pinned", False):
        return
    orig = hw_specs.get_activation_tables
    keep = "natural_log_exp_and_others"

    def patched(arch):
        t = orig(arch)
        return {k: (v if k == keep else set()) for k, v in t.items()}

    bacc.get_activation_tables = patched
    bacc._act_tables_pinned = True


def build_nc(reps=1):
    _pin_act_table_set()
    nc = bacc.Bacc(trn_type="TRN2")
    nc.cur_io = {
        "xT": nc.dram_tensor("xT", [DCH, P, T], BF16, kind="ExternalInput"),
        "wT": nc.dram_tensor("wT", [DCH, P, ETOT], BF16, kind="ExternalInput"),
        "ropeC": nc.dram_tensor("ropeC", [P, T], BF16, kind="ExternalInput"),
        "ropeS": nc.dram_tensor("ropeS", [P, T], BF16, kind="ExternalInput"),
        "tri01": nc.dram_tensor("tri01", [P, KC], BF16, kind="ExternalInput"),
        "yT": nc.dram_tensor("yT", [EQ, T], F32, kind="ExternalOutput"),
    }
    with tile.TileContext(nc) as tc:
        with ExitStack() as ctx:
            su = _setup(nc, tc, ctx)
            _Pipeline(nc, tc, ctx, su, reps).emit()
    nc.finalize()
    return nc


_NC_CACHE = None


def _get_nc():
    global _NC_CACHE
    if _NC_CACHE is None:
        _NC_CACHE = build_nc()
    return _NC_CACHE


def _host_tables():
    inv_freq = 1.0 / (ROPE_BASE ** (np.arange(0, DH, 2, dtype=np.float32) / DH))
    t = np.arange(T, dtype=np.float32)
    freqs = np.outer(t, inv_freq).astype(np.float32)    # (T, 64)
    cosT = np.cos(freqs).T.astype(np.float32)            # (64, T)
    sinT = np.sin(freqs).T.astype(np.float32)
    ropeC = np.concatenate([cosT, cosT], axis=0).astype(NPBF)    # (128, T)
    ropeS = np.concatenate([sinT, -sinT], axis=0).astype(NPBF)
    pp_ = np.arange(KC)[:, None]
    ff = np.arange(KC)[None, :]
    tri01 = np.where(pp_ <= ff, 1.0, 0.0).astype(NPBF)
    return (np.ascontiguousarray(ropeC), np.ascontiguousarray(ropeS),
            np.ascontiguousarray(tri01))


def make_in_maps(x, Wq, Wk, Wv):
    x = np.asarray(x, dtype=np.float32)
    Wq = np.asarray(Wq, dtype=np.float32)
    Wk = np.asarray(Wk, dtype=np.float32)
    Wv = np.asarray(Wv, dtype=np.float32)
    ropeC, ropeS, tri01 = _host_tables()

    xTb = {}
    for b in range(B):
        xTb[b] = np.ascontiguousarray(
            x[b].T.astype(NPBF)).reshape(DCH, P, T)

    in_maps = []
    for core in range(NCORES):
        b, h = divmod(core, HKV)
        Wsl = np.concatenate(
            [
                Wq[h * EQ : (h + 1) * EQ],
                Wk[h * DH : (h + 1) * DH],
                Wv[h * DH : (h + 1) * DH],
            ],
            axis=0,
        )                                                 # (768, D)
        wT = np.ascontiguousarray(Wsl.T.astype(NPBF)).reshape(DCH, P, ETOT)
        in_maps.append(
            {"xT": xTb[b], "wT": wT, "ropeC": ropeC, "ropeS": ropeS,
             "tri01": tri01}
        )
    return in_maps


def kernel(x, Wq, Wk, Wv):
    in_maps = make_in_maps(x, Wq, Wk, Wv)
    nc = _get_nc()
    res = run_bass_kernel_spmd(nc, in_maps, core_ids=list(range(NCORES)))

    out = np.empty((B, T, H * DH), dtype=np.float32)
    for core in range(NCORES):
        b, h = divmod(core, HKV)
        yT = res.results[core]["yT"]                      # (512, T)
        out[b, :, h * EQ : (h + 1) * EQ] = (
            yT.reshape(G, DH, T).transpose(2, 0, 1).reshape(T, EQ)
        )
    return out
